# revision 1
# baseline (speedup 1.0000x reference)
"""InternLM3 self-attention (prefill, GQA, RoPE) on 8 Trainium2 cores.

Tensor-parallel over heads: core r owns q heads 4r..4r+3 and kv head r
(wqkv column shards, wo row shards).  Each core computes its partial
output projection; the 8 partials are summed on the host (an on-device
all-reduce of 32 MB runs at ~32 GB/s through ncfw and would dominate the
kernel, so the reduction is done host-side).

Matmuls run in float32r (TF32-like fast fp32 mode, 1 cycle/row at
N>=512 vs 4 for plain fp32) with fp32 PSUM accumulation.

Device-side layout trick: everything is computed transposed
(qkv^T = wqkv_shard^T @ hidden^T) so that
  - wqkv loads land directly as the stationary operand,
  - q^T/k^T slices feed the scores matmul with head_dim on partitions,
  - scores come out as S^T [k, q], so exp(S^T) feeds the PV matmul
    directly (contraction over k on partitions) with zero transposes,
  - attn^T slices are exactly the stationary operand of the wo matmul.
The only transposes are hidden^T (done host-side, it is an input-layout
choice) and v^T -> v (16 tiny PE transposes).
"""

import numpy as np

import concourse.bass as bass
import concourse.bacc as bacc
import concourse.mybir as mybir
import concourse.tile as tile
from concourse.bass_utils import run_bass_kernel_spmd

T = 2048
H = 4096
NH = 32
NKV = 8
HD = 128
HALF = HD // 2
BASE = 1000000.0
NCORES = 8
QH = NH // NCORES            # 4 q heads per core
QCOLS = QH * HD              # 512
SH_COLS = QCOLS + 2 * HD     # 768 wqkv cols per core
NEG = -1e30

P = 128
TC = 512                     # token chunk (matmul moving dim)
NT = T // TC                 # 4
NHC = H // P                 # 32 contraction chunks for qkv
NQC = SH_COLS // P           # 6 qkv col chunks
NKC = T // P                 # 16 k chunks
NOC = H // TC                # 8 output col chunks
NTC16 = T // P               # 16 token chunks of 128

f32 = mybir.dt.float32
f32r = mybir.dt.float32r

_COMPILED = None


def _build():
    nc = bacc.Bacc("TRN2", target_bir_lowering=False, debug=False,
                   num_devices=NCORES)

    hidT = nc.dram_tensor("hidT", [H, T], f32r, kind="ExternalInput").ap()
    wqkv_s = nc.dram_tensor("wqkv_s", [H, SH_COLS], f32r,
                            kind="ExternalInput").ap()
    wo_s = nc.dram_tensor("wo_s", [QCOLS, H], f32r,
                          kind="ExternalInput").ap()
    cosq = nc.dram_tensor("cosq", [P, T], f32, kind="ExternalInput").ap()
    sinq = nc.dram_tensor("sinq", [P, T], f32, kind="ExternalInput").ap()
    cosk = nc.dram_tensor("cosk", [P, T], f32, kind="ExternalInput").ap()
    sink = nc.dram_tensor("sink", [P, T], f32, kind="ExternalInput").ap()
    masks = nc.dram_tensor("masks", [P, 4, TC], f32,
                           kind="ExternalInput").ap()
    rperm = nc.dram_tensor("rperm", [P, P], f32r, kind="ExternalInput").ap()
    ident = nc.dram_tensor("ident", [P, P], f32r, kind="ExternalInput").ap()
    ones_k = nc.dram_tensor("ones_k", [P, 1], f32r,
                            kind="ExternalInput").ap()
    ones_m = nc.dram_tensor("ones_m", [1, P], f32r,
                            kind="ExternalInput").ap()
    part = nc.dram_tensor("part", [T, H], f32, kind="ExternalOutput").ap()

    with tile.TileContext(nc) as tc:
        with tc.tile_pool(name="keep", bufs=1) as keep:
            # long-lived SBUF: qkv^T [128, 6, 2048] f32r (48 KB/part)
            qkvT = keep.tile([P, NQC, T], f32r)

            # constants first: tiny DMAs, land before the bulk loads
            ct = keep.tile([P, T], f32, tag="cosq_t")
            st = keep.tile([P, T], f32, tag="sinq_t")
            ctk = keep.tile([P, T], f32, tag="cosk_t")
            stk = keep.tile([P, T], f32, tag="sink_t")
            mt = keep.tile([P, 4, TC], f32, tag="masks_t")
            rp = keep.tile([P, P], f32r, tag="rperm_t")
            idt = keep.tile([P, P], f32r, tag="ident_t")
            o_k = keep.tile([P, 1], f32r, tag="ones_k_t")
            o_m = keep.tile([1, P], f32r, tag="ones_m_t")

            # ---------------- phase 1: qkv^T = wqkv^T @ hidden^T -------
            with tc.tile_pool(name="wq", bufs=1) as wqp, \
                 tc.tile_pool(name="hstream", bufs=4) as hsp, \
                 tc.tile_pool(name="qps", bufs=1, space="PSUM") as qpsp:
                wq = wqp.tile([P, NHC, SH_COLS], f32r)
                for h in range(NHC):
                    nc.sync.dma_start(
                        wq[:, h, :], wqkv_s[h * P:(h + 1) * P, :])
                nc.sync.dma_start(ct[:], cosq[:])
                nc.sync.dma_start(st[:], sinq[:])
                nc.sync.dma_start(ctk[:], cosk[:])
                nc.sync.dma_start(stk[:], sink[:])
                nc.sync.dma_start(mt[:], masks[:])
                nc.sync.dma_start(rp[:], rperm[:])
                nc.sync.dma_start(idt[:], ident[:])
                nc.sync.dma_start(o_k[:], ones_k[:])
                nc.sync.dma_start(o_m[:], ones_m[:])
                for t in range(NT):
                    qps = [qpsp.tile([P, TC], f32, tag=f"qps{c}",
                                     name=f"qps{c}_{t}")
                           for c in range(NQC)]
                    for h in range(NHC):
                        ht = hsp.tile([P, TC], f32r, tag="ht")
                        nc.scalar.dma_start(
                            ht[:], hidT[h * P:(h + 1) * P,
                                        t * TC:(t + 1) * TC])
                        for c in range(NQC):
                            nc.tensor.matmul(
                                qps[c][:], wq[:, h, c * P:(c + 1) * P],
                                ht[:], start=(h == 0), stop=(h == NHC - 1))
                    for c in range(NQC):
                        nc.scalar.copy(
                            qkvT[:, c, t * TC:(t + 1) * TC], qps[c][:])

            with tc.tile_pool(name="keep2", bufs=1) as keep2:
                    # ---------------- phase 3: v_nat = v^T transposed ----------
                vnat = keep2.tile([P, NKC, P], f32r, tag="vnat")
                with tc.tile_pool(name="vt_ps", bufs=4, space="PSUM") as vps:
                    for kc in range(NKC):
                        tp = vps.tile([P, P], f32r, tag="vtp")
                        nc.tensor.transpose(
                            tp[:], qkvT[:, 5, kc * P:(kc + 1) * P], idt[:])
                        nc.scalar.copy(vnat[:, kc, :], tp[:])

                # ---------------- phase 2: RoPE on q (scaled) and k --------
                with tc.tile_pool(name="rope_sb", bufs=4) as rsb, \
                     tc.tile_pool(name="rope_ps", bufs=4, space="PSUM") as rps:
                    for idx in range(QH + 1):        # 4 q heads + 1 k head
                        cos_t, sin_t = (ct, st) if idx < QH else (ctk, stk)
                        for t in range(NT):
                            sl = slice(t * TC, (t + 1) * TC)
                            x = qkvT[:, idx, sl]
                            rot = rps.tile([P, TC], f32, tag="rot")
                            nc.tensor.matmul(rot[:], rp[:], x,
                                             start=True, stop=True)
                            tmp = rsb.tile([P, TC], f32, tag="rtmp")
                            nc.vector.tensor_tensor(
                                tmp[:], rot[:], sin_t[:, sl],
                                mybir.AluOpType.mult)
                            nc.vector.tensor_tensor(
                                x, x.bitcast(f32), cos_t[:, sl],
                                mybir.AluOpType.mult)
                            nc.vector.tensor_tensor(
                                x, x.bitcast(f32), tmp[:],
                                mybir.AluOpType.add)

                # ---------------- phase 4: causal attention ----------------
                attnT = keep2.tile([P, QH, T], f32r, tag="attnT")
                with tc.tile_pool(name="att_sb", bufs=8) as asb, \
                     tc.tile_pool(name="att_sm", bufs=4) as asm_p, \
                     tc.tile_pool(name="st_ps", bufs=3, space="PSUM") as stp, \
                     tc.tile_pool(name="pv_ps", bufs=2, space="PSUM") as pvp, \
                     tc.tile_pool(name="d_ps", bufs=2, space="PSUM") as dpp, \
                     tc.tile_pool(name="rb_ps", bufs=1, space="PSUM") as rbp:
                    for head in range(QH):
                        for g in range(NT):
                            kmax = (NT // 1) * (g + 1)   # 4*(g+1) k chunks
                            qsl = slice(g * TC, (g + 1) * TC)
                            d_ps = dpp.tile([1, TC], f32, tag="d")
                            pv = pvp.tile([P, TC], f32, tag="pv")
                            es = asb.tile([P, TC], f32r, tag="esum")
                            e_prev = None
                            for kc in range(kmax):
                                st_ps = stp.tile([P, TC], f32, tag="st")
                                nc.tensor.matmul(
                                    st_ps[:],
                                    qkvT[:, QH, kc * P:(kc + 1) * P],
                                    qkvT[:, head, qsl],
                                    start=True, stop=True)
                                j = kc - 4 * g
                                if j >= 0:
                                    nc.vector.tensor_tensor(
                                        st_ps[:], st_ps[:], mt[:, j, :],
                                        mybir.AluOpType.add)
                                e = asb.tile([P, TC], f32r, tag="E",
                                             name=f"e_{head}_{g}_{kc}")
                                nc.scalar.activation(
                                    e[:], st_ps[:],
                                    mybir.ActivationFunctionType.Exp)
                                # denominator partials on DVE (frees PE)
                                if kc == 1:
                                    nc.vector.tensor_tensor(
                                        es[:], e_prev[:], e[:],
                                        mybir.AluOpType.add)
                                elif kc > 1:
                                    nc.vector.tensor_tensor(
                                        es[:], es[:], e[:],
                                        mybir.AluOpType.add)
                                e_prev = e
                                nc.tensor.matmul(
                                    pv[:], vnat[:, kc, :], e[:],
                                    start=(kc == 0), stop=(kc == kmax - 1))
                            nc.tensor.matmul(d_ps[:], o_k[:], es[:],
                                             start=True, stop=True)
                            rd = asm_p.tile([1, TC], f32, tag="rd")
                            nc.vector.reciprocal(rd[:], d_ps[:])
                            rdr = asm_p.tile([1, TC], f32r, tag="rdr")
                            nc.scalar.copy(rdr[:], rd[:])
                            rb = rbp.tile([P, TC], f32, tag="rb")
                            nc.tensor.matmul(rb[:], o_m[:], rdr[:],
                                             start=True, stop=True)
                            rbs = asm_p.tile([P, TC], f32, tag="rbs")
                            nc.scalar.copy(rbs[:], rb[:])
                            nc.vector.tensor_tensor(
                                attnT[:, head, qsl], pv[:], rbs[:],
                                mybir.AluOpType.mult)

                # ---------------- phase 5: out = attn @ wo_shard -----------
                with tc.tile_pool(name="wo_sb", bufs=3) as wsb, \
                     tc.tile_pool(name="o_sb", bufs=4) as osb, \
                     tc.tile_pool(name="o_ps", bufs=4, space="PSUM") as ops:
                    for oc in range(NOC):
                        wot = wsb.tile([P, QH, TC], f32r, tag="wot")
                        nc.sync.dma_start(
                            wot[:],
                            wo_s[:, oc * TC:(oc + 1) * TC].rearrange(
                                "(hc p) n -> p hc n", p=P))
                        for tcn in range(NTC16):
                            o_ps = ops.tile([P, TC], f32, tag="o")
                            for hc in range(QH):
                                nc.tensor.matmul(
                                    o_ps[:],
                                    attnT[:, hc, tcn * P:(tcn + 1) * P],
                                    wot[:, hc, :],
                                    start=(hc == 0), stop=(hc == QH - 1))
                            ob = osb.tile([P, TC], f32, tag="ob")
                            nc.scalar.copy(ob[:], o_ps[:])
                            nc.gpsimd.dma_start(
                                part[tcn * P:(tcn + 1) * P,
                                     oc * TC:(oc + 1) * TC], ob[:])

    nc.compile()
    return nc


def _rope_tables(positions):
    pos = positions.astype(np.float64)
    inv_freq = 1.0 / (BASE ** (np.arange(HALF, dtype=np.float64) / HALF))
    freqs = pos[:, None] * inv_freq[None, :]          # [T, 64]
    cos = np.cos(freqs)
    sin = np.sin(freqs)
    cosT = np.concatenate([cos, cos], axis=1).T       # [128, T]
    sinT = np.concatenate([-sin, sin], axis=1).T      # sign folded
    return cosT.astype(np.float32), sinT.astype(np.float32)


def kernel(positions, hidden_states, wqkv, wo):
    global _COMPILED
    if _COMPILED is None:
        _COMPILED = _build()
    nc = _COMPILED

    scale = HD ** -0.5
    cosT, sinT = _rope_tables(positions)
    cosq = np.ascontiguousarray(cosT * scale)
    sinq = np.ascontiguousarray(sinT * scale)

    hidT = np.ascontiguousarray(hidden_states.T)

    # causal mask add-tiles for the diagonal blocks, ST layout [k, q]:
    # block j (k chunk 4g+j vs q group g): valid iff 128*j + kl <= ql
    kl = np.arange(P)[:, None]
    ql = np.arange(TC)[None, :]
    masks = np.stack(
        [np.where(P * j + kl <= ql, 0.0, NEG) for j in range(4)],
        axis=1).astype(np.float32)                    # [128, 4, 512]

    rperm = np.zeros((P, P), dtype=np.float32)
    for m in range(P):
        rperm[(m + HALF) % P, m] = 1.0                # out[m]=x[(m+64)%128]
    ident = np.eye(P, dtype=np.float32)
    ones_k = np.ones((P, 1), dtype=np.float32)
    ones_m = np.ones((1, P), dtype=np.float32)

    in_maps = []
    for r in range(NCORES):
        qc = slice(r * QCOLS, (r + 1) * QCOLS)
        kc = slice(NH * HD + r * HD, NH * HD + (r + 1) * HD)
        vc = slice((NH + NKV) * HD + r * HD, (NH + NKV) * HD + (r + 1) * HD)
        wqkv_s = np.ascontiguousarray(
            np.concatenate([wqkv[:, qc], wqkv[:, kc], wqkv[:, vc]], axis=1))
        wo_s = np.ascontiguousarray(wo[qc, :])
        in_maps.append({
            "hidT": hidT, "wqkv_s": wqkv_s, "wo_s": wo_s,
            "cosq": cosq, "sinq": sinq, "cosk": cosT, "sink": sinT,
            "masks": masks, "rperm": rperm, "ident": ident,
            "ones_k": ones_k, "ones_m": ones_m,
        })

    global _LAST_IN_MAPS
    _LAST_IN_MAPS = in_maps
    res = run_bass_kernel_spmd(nc, in_maps, list(range(NCORES)))
    out = res.results[0]["part"].astype(np.float64)
    for r in range(1, NCORES):
        out += res.results[r]["part"]
    return out.astype(np.float32)



# revision 12
# speedup vs baseline: 1.4794x; 1.4794x over previous
"""InternLM3 self-attention (prefill, GQA, RoPE) on 8 Trainium2 cores.

Tensor-parallel over heads: core r owns q heads 4r..4r+3 and kv head r
(wqkv column shards, wo row shards).  Each core computes its partial
output projection; the 8 partials are summed on the host (an on-device
all-reduce of the 32 MB output runs at ~32 GB/s through ncfw and would
dominate the kernel, so the reduction is done host-side).

All matmul operands are float16: same 1 col/cycle PE rate as float32r,
but fast-weight-load kicks in (LDWEIGHTS hidden behind matmuls), DMA
traffic halves, and fp16's 10-bit mantissa keeps rel-err ~1e-3.

Device layout: everything computed transposed (qkv^T = wqkv^T @ hid^T)
so q^T/k^T feed the scores matmul with head_dim on partitions, scores
come out as S^T [k, q] feeding the PV matmul with zero transposes, and
attn^T slices are the stationary operand of the wo matmul.  v is
produced directly in natural [token, hd] layout by swapping the
stationary operand (hid chunk) in phase 1, so no PE transposes at all.
RoPE's rotate-half is an SBUF->SBUF DMA partition shift (no PE perm).

Schedule keeps the PE dense end-to-end (HAM stays warm): phase 1 runs
an 8-bank PSUM ring with zero-stall accumulator recycling; attention
interleaves with the output projection one g-block behind, so the
softmax reciprocal latency hides under wo matmuls.  Scores for the
block-diagonal are trimmed to the causal region at 128-col granularity.
exp() is biased by -ln(16) so fp16 e-values stay far from overflow
(the 16x cancels in the softmax normalization).
"""

import numpy as np

import concourse.bass as bass
import concourse.bacc as bacc
import concourse.mybir as mybir
import concourse.tile as tile
from concourse.bass_utils import run_bass_kernel_spmd

T = 2048
H = 4096
NH = 32
NKV = 8
HD = 128
HALF = HD // 2
BASE = 1000000.0
NCORES = 8
QH = NH // NCORES            # 4 q heads per core
QCOLS = QH * HD              # 512
SH_COLS = QCOLS + 2 * HD     # 768 wqkv cols per core
NEG = -1e30
EXPB = -2.772588722239781    # -ln(16): fp16 overflow headroom for exp

P = 128
TC = 512                     # token chunk (matmul moving dim)
NT = T // TC                 # 4
NHC = H // P                 # 32 contraction chunks for qkv
NQC = QH + 1                 # 5 transposed qkv col chunks (4 q heads + k)
NKC = T // P                 # 16 k chunks
NOC = H // TC                # 8 output col chunks

f32 = mybir.dt.float32
f32r = mybir.dt.float32r
f16 = mybir.dt.float16
MUL = mybir.AluOpType.mult
ADD = mybir.AluOpType.add
EXP = mybir.ActivationFunctionType.Exp

_COMPILED = None


def _build():
    nc = bacc.Bacc("TRN2", target_bir_lowering=False, debug=False,
                   num_devices=NCORES)

    hidT = nc.dram_tensor("hidT", [H, T], f16, kind="ExternalInput").ap()
    wqkv_s = nc.dram_tensor("wqkv_s", [H, SH_COLS], f16,
                            kind="ExternalInput").ap()
    wo_s = nc.dram_tensor("wo_s", [QCOLS, H], f16,
                          kind="ExternalInput").ap()
    cosq = nc.dram_tensor("cosq", [P, T], f16, kind="ExternalInput").ap()
    sinq = nc.dram_tensor("sinq", [P, T], f16, kind="ExternalInput").ap()
    cosk = nc.dram_tensor("cosk", [P, T], f16, kind="ExternalInput").ap()
    sink = nc.dram_tensor("sink", [P, T], f16, kind="ExternalInput").ap()
    mask_d = nc.dram_tensor("mask_d", [P, P], f32,
                            kind="ExternalInput").ap()
    okm_d = nc.dram_tensor("okm_d", [P, QH, QH], f32r,
                           kind="ExternalInput").ap()
    e4_d = nc.dram_tensor("e4_d", [QH, QH, P], f32r,
                          kind="ExternalInput").ap()
    part = nc.dram_tensor("part", [T, H], f16, kind="ExternalOutput").ap()

    with tile.TileContext(nc) as tc:
        with tc.tile_pool(name="keep", bufs=1) as keep:
            # long-lived SBUF
            qkvT = keep.tile([P, NQC, T], f16)       # q0..3,k ^T
            vnat = keep.tile([P, NKC, P], f16)       # v natural [tok, hd]
            attnT = keep.tile([P, QH, T], f16)
            wosb = keep.tile([P, QH, H], f16)
            ct = keep.tile([P, T], f16, tag="ct")
            st = keep.tile([P, T], f16, tag="st_t")
            ctk = keep.tile([P, T], f16, tag="ctk")
            stk = keep.tile([P, T], f16, tag="stk")
            mt = keep.tile([P, P], f32, tag="mt")    # causal triangle
            okt = keep.tile([P, QH, QH], f32r, tag="okt")
            e4t = keep.tile([QH, QH, P], f32r, tag="e4t")
            eb = keep.tile([P, 1], f32, tag="eb")    # exp bias -ln(16)
            nc.gpsimd.memset(eb[:], EXPB)

            # ------------- phase 1: qkv^T = wqkv^T @ hid^T + rope -------
            with tc.tile_pool(name="p1", bufs=1) as p1, \
                 tc.tile_pool(name="ps1", bufs=8, space="PSUM") as ps1:
                wq = p1.tile([P, NHC, SH_COLS], f16)
                # preloads on the scalar-triggered queue
                for h in range(NHC):
                    nc.scalar.dma_start(
                        wq[:, h, :], wqkv_s[h * P:(h + 1) * P, :])
                nc.scalar.dma_start(ct[:], cosq[:])
                nc.scalar.dma_start(st[:], sinq[:])
                nc.scalar.dma_start(ctk[:], cosk[:])
                nc.scalar.dma_start(stk[:], sink[:])
                nc.scalar.dma_start(mt[:], mask_d[:])
                nc.scalar.dma_start(okt[:], okm_d[:])
                nc.scalar.dma_start(e4t[:], e4_d[:])
                nc.scalar.dma_start(
                    wosb[:],
                    wo_s.rearrange("(hc p) n -> p hc n", p=P))

                for t in range(NT):
                    tsl = slice(t * TC, (t + 1) * TC)
                    qps = [ps1.tile([P, TC], f32, tag="ps",
                                    name=f"qps{c}_{t}")
                           for c in range(NQC)]
                    vps = ps1.tile([P, 4, P], f32, tag="ps",
                                   name=f"vps_{t}")
                    for h in range(NHC):
                        ht = p1.tile([P, TC], f16, tag="ht", bufs=6,
                                     name=f"ht_{t}_{h}")
                        nc.sync.dma_start(
                            ht[:], hidT[h * P:(h + 1) * P, tsl])
                        for c in range(NQC):
                            nc.tensor.matmul(
                                qps[c][:], wq[:, h, c * P:(c + 1) * P],
                                ht[:], start=(h == 0), stop=(h == NHC - 1))
                        for s in range(4):
                            # v in natural layout: stationary = hid chunk.
                            # start=True only once per bank: it clears the
                            # whole bank's has_written bits, so the other
                            # sub-regions' first write lands as overwrite.
                            nc.tensor.matmul(
                                vps[:, s, :], ht[:, s * P:(s + 1) * P],
                                wq[:, h, QCOLS + HD:],
                                start=(h == 0 and s == 0),
                                stop=(h == NHC - 1),
                                skip_group_check=not (h == 0 and s == 0))
                    for c in range(NQC):
                        dst = qkvT[:, c, tsl]
                        if c % 2 == 0:
                            nc.scalar.copy(dst, qps[c][:])
                        else:
                            nc.vector.tensor_copy(dst, qps[c][:])
                    nc.scalar.copy(vnat[:, 4 * t:4 * t + 4, :], vps[:])
                    # rope on q heads (scaled) and k for this t-chunk
                    for idx in range(NQC):
                        x = qkvT[:, idx, tsl]
                        cos_t, sin_t = (ct, st) if idx < QH else (ctk, stk)
                        rot = p1.tile([P, TC], f16, tag="rot", bufs=2,
                                      name=f"rot_{t}_{idx}")
                        nc.sync.dma_start(rot[0:HALF, :],
                                          qkvT[HALF:P, idx, tsl])
                        nc.sync.dma_start(rot[HALF:P, :],
                                          qkvT[0:HALF, idx, tsl])
                        tmp = p1.tile([P, TC], f16, tag="rtmp", bufs=2,
                                      name=f"rtmp_{t}_{idx}")
                        nc.vector.tensor_tensor(
                            tmp[:], rot[:], sin_t[:, tsl], MUL)
                        nc.vector.tensor_tensor(x, x, cos_t[:, tsl], MUL)
                        nc.vector.tensor_tensor(x, x, tmp[:], ADD)

            # ------------- phase 2: attention interleaved with wo -------
            with tc.tile_pool(name="att", bufs=1) as att, \
                 tc.tile_pool(name="stp", bufs=3, space="PSUM") as stp, \
                 tc.tile_pool(name="pvp", bufs=2, space="PSUM") as pvp, \
                 tc.tile_pool(name="dpp", bufs=1, space="PSUM") as dpp, \
                 tc.tile_pool(name="opp", bufs=2, space="PSUM") as opp:

                pv_sbs = {}
                rd4s = {}

                def attn_g(g):
                    kmax = 4 * (g + 1)
                    pv_sb = att.tile([P, QH, TC], f32, tag="pvs", bufs=2,
                                     name=f"pvsb_{g}")
                    pv_sbs[g] = pv_sb
                    d_ps4 = dpp.tile([QH, TC], f32, tag="d",
                                     name=f"dps_{g}")

                    def mk_dmm(h, es):
                        def _dmm():
                            nc.tensor.matmul(
                                d_ps4[:], okt[:, h, :],
                                es[:], start=(h == 0), stop=(h == QH - 1))
                        return _dmm

                    prev_dmm = None
                    for h in range(QH):
                        pv = pvp.tile([P, TC], f32, tag="pv",
                                      name=f"pv_{g}_{h}")
                        es = att.tile([P, TC], f32r, tag="es", bufs=2,
                                      name=f"es_{g}_{h}")
                        e0 = None
                        for kc in range(kmax):
                            j = kc - 4 * g
                            q0 = P * j if j > 0 else 0
                            W = TC - q0
                            stt = stp.tile([P, TC], f32, tag="stps",
                                           name=f"st_{g}_{h}_{kc}")
                            nc.tensor.matmul(
                                stt[:, 0:W],
                                qkvT[:, QH, kc * P:(kc + 1) * P],
                                qkvT[:, h, g * TC + q0:(g + 1) * TC],
                                start=True, stop=True)
                            if kc == 1 and prev_dmm is not None:
                                prev_dmm()   # deferred: PE filler
                                prev_dmm = None
                            if j >= 0:
                                nc.vector.tensor_tensor(
                                    stt[:, 0:P], stt[:, 0:P], mt[:], ADD)
                            e = att.tile([P, TC], f16, tag="e", bufs=6,
                                         name=f"e_{g}_{h}_{kc}")
                            nc.scalar.activation(
                                e[:, 0:W], stt[:, 0:W], EXP, bias=eb[:])
                            if kc == 0:
                                e0 = e
                            elif kc == 1:
                                if g == 0:   # e1 starts at col 128
                                    nc.vector.tensor_tensor(
                                        es[:, P:], e0[:, P:],
                                        e[:, 0:TC - P], ADD)
                                    nc.vector.tensor_copy(
                                        es[:, 0:P], e0[:, 0:P])
                                else:
                                    nc.vector.tensor_tensor(
                                        es[:], e0[:], e[:], ADD)
                            else:
                                nc.vector.tensor_tensor(
                                    es[:, q0:], es[:, q0:], e[:, 0:W], ADD)
                            nc.tensor.matmul(
                                pv[:, q0:], vnat[:, kc, :], e[:, 0:W],
                                start=(kc == 0), stop=(kc == kmax - 1),
                                skip_group_check=(q0 > 0))
                        nc.vector.tensor_copy(pv_sb[:, h, :], pv[:])
                        prev_dmm = mk_dmm(h, es)
                    return prev_dmm, d_ps4

                def recip_g(g, last_dmm, d_ps4):
                    last_dmm()
                    dsb = att.tile([QH, TC], f32, tag="dsb", bufs=2,
                                   name=f"dsb_{g}")
                    nc.scalar.copy(dsb[:], d_ps4[:])
                    rd4 = att.tile([QH, TC], f32, tag="rd", bufs=2,
                                   name=f"rd_{g}")
                    nc.vector.reciprocal(rd4[:], dsb[:])
                    rd4r = att.tile([QH, TC], f32r, tag="rdr", bufs=2,
                                    name=f"rdr_{g}")
                    nc.scalar.copy(rd4r[:], rd4[:])
                    rd4s[g] = rd4r

                def norm_g(g):
                    rd4 = rd4s[g]
                    for h in range(QH):
                        # rb shares the pv PSUM ring (see bank budget)
                        rb = pvp.tile([P, TC], f32, tag="pv",
                                      name=f"rb_{g}_{h}")
                        nc.tensor.matmul(
                            rb[:], e4t[:, h, :], rd4[:],
                            start=True, stop=True)
                        rbs = att.tile([P, TC], f32, tag="rbs", bufs=2,
                                       name=f"rbs_{g}_{h}")
                        nc.scalar.copy(rbs[:], rb[:])
                        nc.vector.tensor_tensor(
                            attnT[:, h, g * TC:(g + 1) * TC],
                            pv_sbs[g][:, h, :], rbs[:], MUL)

                def wo_block(g):
                    for tcn in range(4 * g, 4 * g + 4):
                        for oc in range(NOC):
                            o_ps = opp.tile([P, TC], f32, tag="o",
                                            name=f"o_{tcn}_{oc}")
                            for hc in range(QH):
                                nc.tensor.matmul(
                                    o_ps[:],
                                    attnT[:, hc, tcn * P:(tcn + 1) * P],
                                    wosb[:, hc, oc * TC:(oc + 1) * TC],
                                    start=(hc == 0), stop=(hc == QH - 1))
                            ob = att.tile([P, TC], f16, tag="ob", bufs=4,
                                          name=f"ob_{tcn}_{oc}")
                            if (tcn + oc) % 2 == 0:
                                nc.vector.tensor_copy(ob[:], o_ps[:])
                            else:
                                nc.scalar.copy(ob[:], o_ps[:])
                            nc.gpsimd.dma_start(
                                part[tcn * P:(tcn + 1) * P,
                                     oc * TC:(oc + 1) * TC], ob[:])

                # software pipeline: wo one g-block behind attention, so
                # the reciprocal latency hides under wo matmuls
                pend = {}
                for g in range(NT):
                    pend[g] = attn_g(g)
                    if g > 0:
                        norm_g(g - 1)
                    recip_g(g, *pend[g])
                    if g > 0:
                        wo_block(g - 1)
                norm_g(NT - 1)
                wo_block(NT - 1)

    nc.compile()
    return nc


def _rope_tables(positions):
    pos = positions.astype(np.float64)
    inv_freq = 1.0 / (BASE ** (np.arange(HALF, dtype=np.float64) / HALF))
    freqs = pos[:, None] * inv_freq[None, :]          # [T, 64]
    cos = np.cos(freqs)
    sin = np.sin(freqs)
    cosT = np.concatenate([cos, cos], axis=1).T       # [128, T]
    sinT = np.concatenate([-sin, sin], axis=1).T      # sign folded
    return cosT, sinT


def kernel(positions, hidden_states, wqkv, wo):
    global _COMPILED
    if _COMPILED is None:
        _COMPILED = _build()
    nc = _COMPILED

    scale = HD ** -0.5
    cosT, sinT = _rope_tables(positions)
    cosq_h = (cosT * scale).astype(np.float16)
    sinq_h = (sinT * scale).astype(np.float16)
    cosk_h = cosT.astype(np.float16)
    sink_h = sinT.astype(np.float16)

    hidT = np.ascontiguousarray(hidden_states.T).astype(np.float16)

    # causal triangle for a 128x128 diagonal block, ST layout [k, q]
    kl = np.arange(P)[:, None]
    ql = np.arange(P)[None, :]
    mask = np.where(kl <= ql, 0.0, NEG).astype(np.float32)

    # one-hot stats: okm reduces es over partitions into row h;
    # e4 broadcasts rd4 row h to all 128 partitions
    okm = np.zeros((P, QH, QH), dtype=np.float32)
    for h in range(QH):
        okm[:, h, h] = 1.0
    e4 = np.zeros((QH, QH, P), dtype=np.float32)
    for h in range(QH):
        e4[h, h, :] = 1.0

    in_maps = []
    for r in range(NCORES):
        qc = slice(r * QCOLS, (r + 1) * QCOLS)
        kc = slice(NH * HD + r * HD, NH * HD + (r + 1) * HD)
        vc = slice((NH + NKV) * HD + r * HD, (NH + NKV) * HD + (r + 1) * HD)
        wqkv_s = np.ascontiguousarray(np.concatenate(
            [wqkv[:, qc], wqkv[:, kc], wqkv[:, vc]],
            axis=1)).astype(np.float16)
        wo_s = np.ascontiguousarray(wo[qc, :]).astype(np.float16)
        in_maps.append({
            "hidT": hidT, "wqkv_s": wqkv_s, "wo_s": wo_s,
            "cosq": cosq_h, "sinq": sinq_h, "cosk": cosk_h, "sink": sink_h,
            "mask_d": mask, "okm_d": okm, "e4_d": e4,
        })

    global _LAST_IN_MAPS
    _LAST_IN_MAPS = in_maps
    res = run_bass_kernel_spmd(nc, in_maps, list(range(NCORES)))
    out = res.results[0]["part"].astype(np.float32)
    for r in range(1, NCORES):
        out += res.results[r]["part"]
    return out.astype(np.float32)
